# revision 38
# baseline (speedup 1.0000x reference)
"""GCN encoder (2x GCNConv + ReLU + AdaptiveAvgPool) on 8 Trainium2 NeuronCores.

Math (matches reference):
    deg[i]  = #edges with dst==i (+1 self loop);  dinv = deg^-1/2
    h       = relu( A_norm @ (x @ W1) + b1 ),  A_norm = D^-1/2 (A+I) D^-1/2
    out2    = A_norm @ (h @ W2) + b2
    pooled[g] = mean over nodes n in group g (1600 nodes) of out2[n]

Algebraic restructurings (exact, fp-reassociation only):
  * W1 commutes with aggregation: per-edge payload is one 16-float x row.
  * A_norm factorizes: agg[d] = dinv[d] * sum_{e->d} xd[src_e], xd = dinv*x.
  * pooled needs only z[g] = sum_n C[n,g] * (dinv[n] h[n]) with host-built
    C[n,g] = sum_{e: src=n, dst in g} dinv[dst].  pooled = (z @ W2)/1600 + b2.

HW facts (measured): dma_gather is DESCRIPTOR-rate-limited at ~2.05 ns per
gathered element on 4 SWDGE queues, independent of element size 64B-512B;
indirect-DMA/scatter-add/SBUF-source paths are no faster and don't compose.
So minimize gathered-element count and keep everything else off the
critical path:
  * bf16 quad table (4 nodes x 16 feats = 128B payload in a 256B-stride row,
    quad index src>>2 < 12800 fits int16); elem_size 128B needs a local
    clone of dma_gather without its elem%256 assert (stride is still 256B).
  * (dst, quad) slots are DEDUPED: parallel edges and same-quad sources
    merge into one slot, their dinv weights accumulate in the mask.
  * self-loops never enter the gather: host streams sxd = dinv^2 * x rows
    and the kernel adds them to the aggregation directly.
  * per-slot mask (bf16, dinv[dst] x onehot(src&3)) selects the sub-row
    during a fused DVE multiply + free-axis segment reduce.
  * the whole idx + mask tables fit in SBUF and load once as constants;
    nothing streams against the gathers.  Chunk counts are tuned to a
    multiple of 4 so equal-size gathers round-robin the 4 queues evenly.
Device work per core: ~101k-slot gather (~13 MB random 128B reads),
mask-mult + segment reduce, transpose, @W1+b1, relu*dinv, z psum-accum,
(z@W2+200*b2)/1600.  Host combines: output = sum of 8 partial p_out.
"""

import numpy as np

N = 51200
E = 819200
F = 16          # input feats
H1 = 64         # hidden
H2 = 128        # output feats
G = 32          # pool groups
GS = N // G     # 1600 nodes per group
NCORES = 8
NPC = N // NCORES       # nodes per core: 6400
NBLK = NPC // 128       # 50 blocks of 128 nodes
PAD_IDX = 10_000_000    # host-side empty-slot marker
GCOLS = 64              # max slot columns per dma_gather (8192 idxs)
CHUNK_G = 2             # gathers per chunk
CHUNK_COLS = 128        # max slot columns per chunk


def _quad_match(src, dst):
    """Renumber nodes so sources co-cited by the same dst share quads:
    co-cited quad members merge into ONE gather slot (the 4-wide mask is
    multi-hot), cutting gathered-element count ~6%.  Greedy maximal
    matching on the co-citation graph (nodes->pairs), then again
    (pairs->quads).  Returns newpos[old_node] = new position."""
    rng = np.random.default_rng(0)
    order = np.argsort(dst, kind="stable")
    ds, ss = dst[order], src[order]

    def pair_events(ids):
        out, k = [], 1
        while True:
            same = ds[:-k] == ds[k:]
            if not same.any():
                break
            a, b = ids[:-k][same], ids[k:][same]
            lo, hi = np.minimum(a, b), np.maximum(a, b)
            ok = lo != hi
            out.append(lo[ok].astype(np.int64) * (1 << 20) + hi[ok])
            k += 1
        return np.concatenate(out)

    def match(u, v, w, nn, used, maxpass=15):
        key = w.astype(np.float64) + rng.random(len(w))
        o = np.argsort(-key, kind="stable")
        u, v = u[o], v[o]
        su_all, sv_all = [], []
        for _ in range(maxpass):
            ok = ~used[u] & ~used[v]
            u, v = u[ok], v[ok]
            if len(u) == 0:
                break
            pos = np.arange(len(u))
            first = np.full(nn, 1 << 60, np.int64)
            np.minimum.at(first, u, pos)
            np.minimum.at(first, v, pos)
            take = (first[u] == pos) & (first[v] == pos)
            used[u[take]] = True
            used[v[take]] = True
            su_all.append(u[take])
            sv_all.append(v[take])
            u, v = u[~take], v[~take]
        return np.concatenate(su_all), np.concatenate(sv_all)

    uq, w = np.unique(pair_events(ss), return_counts=True)
    used = np.zeros(N, bool)
    pu, pv = match(uq >> 20, uq & ((1 << 20) - 1), w, N, used)
    P = len(pu)
    snode = np.full(N, -1, np.int64)
    snode[pu] = np.arange(P)
    snode[pv] = np.arange(P)
    singles = np.where(snode < 0)[0]
    snode[singles] = P + np.arange(len(singles))
    NS = P + len(singles)
    uq2, w2 = np.unique(pair_events(snode[ss]), return_counts=True)
    u2, v2 = uq2 >> 20, uq2 & ((1 << 20) - 1)
    bp = (u2 < P) & (v2 < P)
    used2 = np.zeros(NS, bool)
    qu, qv = match(u2[bp], v2[bp], w2[bp], NS, used2)

    quads = [[pu[a], pv[a], pu[b], pv[b]] for a, b in zip(qu, qv)]
    left = np.where(~used2[:P])[0]
    for i in range(0, len(left) - 1, 2):
        a, b = left[i], left[i + 1]
        quads.append([pu[a], pv[a], pu[b], pv[b]])
    tail = ([pu[left[-1]], pv[left[-1]]] if len(left) % 2 else []) \
        + list(singles)
    for i in range(0, len(tail), 4):
        quads.append(tail[i:i + 4])
    newpos = np.full(N, -1, np.int64)
    p = 0
    for q in quads:
        for n in q:
            newpos[n] = p
            p += 1
    assert p == N and (newpos >= 0).all()
    return newpos


def _prep(x, edge_index, W1, b1, W2, b2):
    """Host-side graph preprocessing: degrees, norms, C matrix, per-core
    deduped quad-slot + mask tables.  Returns (static_cfg, per_core_inmaps)."""
    import ml_dtypes
    bf16 = ml_dtypes.bfloat16

    src = edge_index[0].astype(np.int64)
    dst = edge_index[1].astype(np.int64)

    deg_e = np.bincount(dst, minlength=N)           # edge in-degree
    deg = deg_e + 1                                 # + self loop
    dinv = (1.0 / np.sqrt(deg.astype(np.float64))).astype(np.float32)

    xd = (x.astype(np.float32) * dinv[:, None]).astype(np.float32)
    newpos = _quad_match(src, dst)          # co-citation-aware quad layout
    inv_perm = np.argsort(newpos)           # new position -> old node
    xdq = np.zeros((N // 4 + 1, 8 * F), bf16)       # 256B rows, 128B payload
    xdq[:N // 4, :4 * F] = xd[inv_perm].reshape(N // 4, 4 * F).astype(bf16)

    # C[n, g] = sum_{e: src=n, dst//GS=g} dinv[dst]  (+ self loop term)
    g_e = dst // GS
    C = np.bincount(src * G + g_e, weights=dinv[dst].astype(np.float64),
                    minlength=N * G).astype(np.float32).reshape(N, G)
    C[np.arange(N), np.arange(N) // GS] += dinv

    # deduped (dst, quad) slots with per-subrow accumulated dinv weights
    ps = newpos[src]
    keys = (dst << 14) | (ps >> 2)
    uk, inv = np.unique(keys, return_inverse=True)
    nslot = uk.shape[0]
    mvals = np.zeros((nslot, 4), np.float32)
    np.add.at(mvals, (inv, ps & 3), dinv[dst])
    u_dst = (uk >> 14).astype(np.int64)
    u_quad = (uk & 16383).astype(np.int32)
    nsl = np.bincount(u_dst, minlength=N)           # slots per node
    row_start = np.zeros(N + 1, np.int64)
    np.cumsum(nsl, out=row_start[1:])

    # slot table per node, padded per 128-node block
    maxsl = int(nsl.max())
    Tq = np.full((N, maxsl), N // 4, np.int32)      # pad -> zeros row
    Tm = np.zeros((N, maxsl, 4), np.float32)
    cols = (np.arange(nslot) - row_start[u_dst])
    Tq[u_dst, cols] = u_quad
    Tm[u_dst, cols] = mvals

    # degree-sorted, strided node->core assignment (same cap profile on
    # every core -> one SPMD program)
    order_n = np.argsort(nsl, kind="stable")
    cores_nodes = [order_n[c::NCORES] for c in range(NCORES)]

    caps = []
    for B in range(NBLK):
        m = 1
        for c in range(NCORES):
            nodes = cores_nodes[c][B * 128:(B + 1) * 128]
            m = max(m, int(nsl[nodes].max()))
        caps.append(m)

    # chunk assignment: whole blocks per chunk, <= `bound` slot columns.
    # Pick the largest bound <= CHUNK_COLS whose chunk count is a multiple
    # of 4, so the CHUNK_G equal gathers per chunk round-robin onto the 4
    # SWDGE queues with identical per-queue element totals.
    def mk_chunks(bound):
        ch = []
        col0, b0, acc = 0, 0, 0
        for B in range(NBLK):
            if acc + caps[B] > bound:
                ch.append((b0, B, col0))
                col0 += bound
                b0, acc = B, 0
            acc += caps[B]
        ch.append((b0, NBLK, col0))
        return ch

    cbound = CHUNK_COLS
    for bound in range(CHUNK_COLS, CHUNK_COLS // 2, -1):
        if len(mk_chunks(bound)) % 4 == 0:
            cbound = bound
            break
    chunks = mk_chunks(cbound)
    SP = chunks[-1][2] + cbound     # padded total slot columns
    boff = {}
    for (bb0, bb1, c0) in chunks:
        c = c0
        for B in range(bb0, bb1):
            boff[B] = c
            c += caps[B]

    # block-diagonal W1 (8 copies) so a whole chunk's aggT goes through one
    # matmul; b1 tiled to match
    w1bd = np.zeros((128, 8 * H1), np.float32)
    for b in range(8):
        w1bd[b * F:(b + 1) * F, b * H1:(b + 1) * H1] = W1.astype(np.float32)
    b1rep = np.tile(b1.astype(np.float32), 8).reshape(1, 8 * H1)
    w2 = np.ascontiguousarray(W2.astype(np.float32))
    b2r = np.ascontiguousarray(b2.astype(np.float32).reshape(1, H2))

    W16 = GCOLS * 128 // 16      # 512 idx columns per gather
    per_core = []
    for c in range(NCORES):
        qidx_cols = np.full((128, SP), N // 4, np.int32)
        mask_all = np.zeros((128, SP, 4), np.float32)
        dinv_pos = np.zeros((128, NBLK), np.float32)
        c_all = np.zeros((128, NBLK * G), np.float32)
        sxd = np.zeros((128, NBLK * F), np.float32)
        for B in range(NBLK):
            nodes = cores_nodes[c][B * 128:(B + 1) * 128]
            cap = caps[B]
            o = boff[B]
            qidx_cols[:, o:o + cap] = Tq[nodes, :cap]
            mask_all[:, o:o + cap, :] = Tm[nodes, :cap]
            dinv_pos[:, B] = dinv[nodes]
            # dinv (layer-2 src norm) folded into C: z-matmul needs no
            # per-partition scale, so the relu batches per chunk
            c_all[:, B * G:(B + 1) * G] = C[nodes] * dinv[nodes][:, None]
            sxd[:, B * F:(B + 1) * F] = xd[nodes] * dinv[nodes][:, None]
        # per-gather wrap-16 int16 index streams, replicated to 128 parts.
        # Each chunk's VALID columns (block caps are packed back-to-back
        # from its base) are split into CHUNK_G equal gathers -> all
        # gathers are nearly the same size and round-robin across the 4
        # queues stays balanced at every pipeline instant.
        gathers = []     # (chunk_i, local_colstart, ncols, idx_col_offset)
        parts = []
        icol = 0
        for cidx, (bb0, bb1, cc0) in enumerate(chunks):
            v = boff[bb1 - 1] + caps[bb1 - 1] - cc0   # valid cols in chunk
            s0 = 0
            for gi in range(CHUNK_G):
                ncols = (v + CHUNK_G - 1 - gi) // CHUNK_G
                if ncols == 0:
                    continue
                pos = qidx_cols[:, cc0 + s0:cc0 + s0 + ncols]
                pv = pos.T.ravel()                     # position order
                parts.append(pv.reshape(ncols * 8, 16).T.astype(np.int16))
                gathers.append((cidx, s0, ncols, icol))
                s0 += ncols
                icol += ncols * 8
        gidx = np.tile(np.concatenate(parts, axis=1), (8, 1))
        per_core.append(dict(
            xdq=xdq, gidx=gidx,
            mask_all=mask_all.reshape(128, SP * 4).astype(bf16),
            c_all=c_all, sxd=sxd,
            w1bd=w1bd, b1rep=b1rep, w2=w2, b2r=b2r,
        ))

    cfg = (tuple(caps), tuple(chunks), SP, tuple(gathers), icol, cbound)
    return cfg, per_core


def _small_elem_gather(eng, out_ap, in_ap, idxs_ap, num_idxs, num_idxs_reg,
                       elem_size, elem_step, queue_num):
    """dma_gather clone without the elem_size_bytes%256 assert; the encoded
    stride (elem_step) must still be a 256B multiple."""
    from concourse import ap_utils, mybir
    from concourse.bass import MemorySpace
    eng._assert_queue_num(queue_num)
    assert idxs_ap.dtype == mybir.dt.int16
    assert in_ap.space == MemorySpace.DRAM
    assert in_ap.dtype == out_ap.dtype
    assert ap_utils.ap_is_contiguous(out_ap.ap[1:])
    assert ap_utils.ap_is_contiguous(idxs_ap.ap[1:])
    assert in_ap.ap[-1][1] == out_ap.ap[-1][1] == elem_size
    assert in_ap.ap[0][0] == elem_step
    stride_bytes = elem_step * mybir.dt.size(in_ap.dtype)
    stride_bytes_256 = stride_bytes // 256
    assert stride_bytes % 256 == 0 and stride_bytes_256 < 256
    _in_ap = eng.lower_ap_dma(in_ap, for_custom_bir_dma=True)
    _idxs_ap = eng.lower_ap(idxs_ap)
    _out_ap = eng.lower_ap(out_ap)
    return eng.add_instruction(
        mybir.InstDMAGatherAnt(
            name=eng.bass.get_next_instruction_name(),
            ins=[*_in_ap, _idxs_ap,
                 eng.lower_val_access(eng.to_reg(num_idxs_reg))],
            outs=[_out_ap],
            transpose=False,
            num_idxs=num_idxs,
            elem_size=elem_size,
            stride_bytes_256=stride_bytes_256,
            gen_mode=0,
            single_packet=False,
            queue_num=queue_num,
            sbuf_tokens_per_rank=0,
            sbuf_free_dim_per_rank=0,
            sbuf_free_dim_pad_per_rank=0,
            sbuf_byte_offset=0,
        ))


def _build(cfg, nrep=1, ablate=None):
    # nrep > 1 duplicates the pipeline body (timing only; output invalid).
    # ablate: None | "gather" (skip compute) | "compute" (skip gathers) —
    # timing-only in-situ profiling since NTFF tracing is unavailable.
    import concourse.bass as bass
    import concourse.bacc as bacc
    import concourse.tile as tile
    from concourse import mybir
    from concourse.masks import make_identity

    caps, chunks, SP, gathers, idxcols, cbound = cfg
    boff = {}
    for (bb0, bb1, c0) in chunks:
        c = c0
        for B in range(bb0, bb1):
            boff[B] = c
            c += caps[B]

    f32 = mybir.dt.float32
    bf16 = mybir.dt.bfloat16
    i16 = mybir.dt.int16

    nc = bacc.Bacc("TRN2", target_bir_lowering=False, debug=False,
                   num_devices=NCORES, num_swdge_queues=4)

    xdq_t = nc.dram_tensor("xdq", [N // 4 + 1, 8 * F], bf16,
                           kind="ExternalInput")
    gidx_t = nc.dram_tensor("gidx", [128, idxcols], i16,
                            kind="ExternalInput")
    mask_t = nc.dram_tensor("mask_all", [128, SP * 4], bf16,
                            kind="ExternalInput")
    c_t = nc.dram_tensor("c_all", [128, NBLK * G], f32, kind="ExternalInput")
    sxd_t = nc.dram_tensor("sxd", [128, NBLK * F], f32, kind="ExternalInput")
    w1bd_t = nc.dram_tensor("w1bd", [128, 8 * H1], f32, kind="ExternalInput")
    b1_t = nc.dram_tensor("b1rep", [1, 8 * H1], f32, kind="ExternalInput")
    w2_t = nc.dram_tensor("w2", [H1, H2], f32, kind="ExternalInput")
    b2_t = nc.dram_tensor("b2r", [1, H2], f32, kind="ExternalInput")
    out_t = nc.dram_tensor("p_out", [G, H2], f32, kind="ExternalOutput")

    by_chunk = {}
    for (cidx, s0, ncols, icol) in gathers:
        by_chunk.setdefault(cidx, []).append((s0, ncols, icol))

    AF = mybir.ActivationFunctionType
    AX = mybir.AxisListType
    OP = mybir.AluOpType

    with tile.TileContext(nc) as tc:
        with tc.tile_pool(name="const", bufs=1) as constp, \
             tc.tile_pool(name="stream", bufs=2) as streamp, \
             tc.tile_pool(name="work", bufs=3) as workp, \
             tc.tile_pool(name="psum", bufs=2, space="PSUM") as psump, \
             tc.tile_pool(name="psumacc", bufs=1, space="PSUM") as psumaccp:

            ident = constp.tile([128, 128], f32)
            make_identity(nc, ident[:])
            ones_row = constp.tile([1, 128], f32)
            nc.vector.memset(ones_row[:], 1.0)
            ones_b2 = constp.tile([1, G], f32)
            nc.vector.memset(ones_b2[:], float(GS) / NCORES)   # 200.0

            w1bd = constp.tile([128, 8 * H1], f32)
            nc.sync.dma_start(out=w1bd[:], in_=w1bd_t[:, :])
            b1s = constp.tile([1, 8 * H1], f32)
            nc.sync.dma_start(out=b1s[:], in_=b1_t[:, :])
            w2 = constp.tile([H1, H2], f32)
            nc.sync.dma_start(out=w2[:], in_=w2_t[:, :])
            b2s = constp.tile([1, H2], f32)
            nc.sync.dma_start(out=b2s[:], in_=b2_t[:, :])
            call = constp.tile([128, NBLK * G], f32)
            nc.sync.dma_start(out=call[:], in_=c_t[:, :])
            sxd = constp.tile([128, NBLK * F], f32)
            nc.sync.dma_start(out=sxd[:], in_=sxd_t[:, :])
            # whole idx + mask tables fit in SBUF: load once, no per-rep
            # streaming competes with the gathers
            idxall = constp.tile([128, idxcols], i16)
            nc.sync.dma_start(out=idxall[:], in_=gidx_t[:, :])
            maskall = constp.tile([128, SP * 4], bf16)
            nc.scalar.dma_start(out=maskall[:], in_=mask_t[:, :])

            psum_zT = psumaccp.tile([H1, G], f32)

            gq = 0
            rep_chunks = [c for _ in range(nrep) for c in chunks]
            for ci, (b0, b1_, c0) in enumerate(rep_chunks):
                cidx = ci % len(chunks)
                gchunk = streamp.tile([128, cbound * 4 * F], bf16,
                                      tag="gbuf", bufs=6)
                if ablate == "compute":
                    nc.gpsimd.memset(gchunk[:], 0.0)
                else:
                    for (s0, ncols, icol) in by_chunk[cidx]:
                        _small_elem_gather(
                            nc.gpsimd,
                            out_ap=gchunk[:, s0 * 4 * F:
                                          (s0 + ncols) * 4 * F].rearrange(
                                "p (s f) -> p s f", f=4 * F),
                            in_ap=xdq_t[:, :4 * F],
                            idxs_ap=idxall[:, icol:icol + ncols * 8],
                            num_idxs=ncols * 128,
                            num_idxs_reg=ncols * 128,
                            elem_size=4 * F, elem_step=8 * F,
                            queue_num=gq % 4)
                        gq += 1
                if ablate == "gather":
                    # consume one tiny slice of EACH gather so the final
                    # output depends on every DMA (forces full drain) while
                    # adding negligible DVE/PE work
                    agg = workp.tile([128, F], f32, tag="agg")
                    for (s0, ncols, icol) in by_chunk[cidx]:
                        nc.vector.tensor_reduce(
                            agg[:],
                            gchunk[:, s0 * 4 * F:(s0 + 1) * 4 * F].rearrange(
                                "p (s f) -> p f s", f=F),
                            axis=AX.X, op=OP.add)
                    nc.tensor.matmul(out=psum_zT[:F, :], lhsT=agg[:],
                                     rhs=call[:, :G],
                                     start=(ci == 0),
                                     stop=(ci == len(rep_chunks) - 1),
                                     skip_group_check=True)
                    continue
                for B in range(b0, b1_):
                    o = boff[B] - c0
                    cap = caps[B]
                    gv = gchunk[:, o * 4 * F:(o + cap) * 4 * F].rearrange(
                        "p (c j f) -> p c j f", j=4, f=F)
                    mv = maskall[:, (c0 + o) * 4:(c0 + o + cap) * 4].rearrange(
                        "p (c j) -> p c j", j=4)
                    mvb = bass.AP(mv.tensor, mv.offset,
                                  list(mv.ap) + [[0, F]])
                    # in-place: select the right sub-row and fold dinv[dst]
                    nc.vector.tensor_tensor(out=gv, in0=gv, in1=mvb,
                                            op=OP.mult)
                    agg = workp.tile([128, F], f32, tag="agg")
                    nc.vector.tensor_reduce(
                        agg[:],
                        gchunk[:, o * 4 * F:(o + cap) * 4 * F].rearrange(
                            "p (s f) -> p f s", f=F),
                        axis=AX.X, op=OP.add)
                    # self-loop term, host-prescaled: agg += dinv^2 * x
                    nc.vector.tensor_tensor(
                        out=agg[:], in0=agg[:],
                        in1=sxd[:, B * F:(B + 1) * F], op=OP.add)
                    pt = psump.tile([F, 128], f32, tag="pt")
                    nc.tensor.transpose(out=pt[:], in_=agg[:],
                                        identity=ident[:])
                    aggT = workp.tile([F, 128], f32, tag="aggT")
                    nc.scalar.copy(aggT[:], pt[:])
                    ph = psump.tile([128, H1], f32, tag="ph")
                    nc.tensor.matmul(out=ph[:], lhsT=aggT[:],
                                     rhs=w1bd[:F, :H1],
                                     start=True, stop=False)
                    nc.tensor.matmul(out=ph[:], lhsT=ones_row[:],
                                     rhs=b1s[:, :H1],
                                     start=False, stop=True)
                    hd = workp.tile([128, H1], f32, tag="hd")
                    nc.scalar.activation(hd[:], ph[:], AF.Relu)
                    nc.tensor.matmul(out=psum_zT[:], lhsT=hd[:],
                                     rhs=call[:, B * G:(B + 1) * G],
                                     start=(ci == 0 and B == b0),
                                     stop=(ci == len(rep_chunks) - 1
                                           and B == b1_ - 1),
                                     skip_group_check=True)

            zT = constp.tile([H1, G], f32)
            nc.scalar.copy(zT[:], psum_zT[:])
            pP = psump.tile([G, H2], f32, tag="pP")
            nc.tensor.matmul(out=pP[:], lhsT=zT[:], rhs=w2[:],
                             start=True, stop=False)
            nc.tensor.matmul(out=pP[:], lhsT=ones_b2[:], rhs=b2s[:],
                             start=False, stop=True)
            pout = constp.tile([G, H2], f32)
            nc.scalar.activation(pout[:], pP[:], AF.Copy, scale=1.0 / GS)
            nc.sync.dma_start(out=out_t[:, :], in_=pout[:])

    nc.compile()
    return nc


_CACHE = {}


def kernel(**inputs):
    x = np.asarray(inputs["x"], dtype=np.float32)
    edge_index = np.asarray(inputs["edge_index"])
    W1 = np.asarray(inputs["W1"], dtype=np.float32)
    b1 = np.asarray(inputs["b1"], dtype=np.float32)
    W2 = np.asarray(inputs["W2"], dtype=np.float32)
    b2 = np.asarray(inputs["b2"], dtype=np.float32)
    assert x.shape == (N, F) and edge_index.shape == (2, E)

    cfg, per_core = _prep(x, edge_index, W1, b1, W2, b2)

    from concourse.bass_utils import run_bass_kernel_spmd

    if cfg not in _CACHE:
        _CACHE[cfg] = _build(cfg)
    nc = _CACHE[cfg]

    res = run_bass_kernel_spmd(nc, per_core, list(range(NCORES)))
    out = np.zeros((G, H2), np.float64)
    for r in res.results:
        out += r["p_out"].astype(np.float64)
    return out.astype(np.float32).reshape(1, G, H2)


if __name__ == "__main__":
    rng = np.random.default_rng(0)
    ins = dict(
        x=rng.standard_normal((N, F), dtype=np.float32),
        edge_index=rng.integers(0, N, (2, E)).astype(np.int32),
        W1=rng.standard_normal((F, H1), dtype=np.float32) * 0.25,
        b1=np.zeros(H1, np.float32),
        W2=rng.standard_normal((H1, H2), dtype=np.float32) * 0.125,
        b2=np.zeros(H2, np.float32),
    )
    out = kernel(**ins)
    print(out.shape, out.dtype, float(np.abs(out).mean()))


# revision 39
# speedup vs baseline: 1.3272x; 1.3272x over previous
"""GCN encoder (2x GCNConv + ReLU + AdaptiveAvgPool) on 8 Trainium2 NeuronCores.

Math (matches reference):
    deg[i]  = #edges with dst==i (+1 self loop);  dinv = deg^-1/2
    h       = relu( A_norm @ (x @ W1) + b1 ),  A_norm = D^-1/2 (A+I) D^-1/2
    out2    = A_norm @ (h @ W2) + b2
    pooled[g] = mean over nodes n in group g (1600 nodes) of out2[n]

Algebraic restructurings (exact, fp-reassociation only):
  * W1 commutes with aggregation: per-edge payload is one 16-float x row.
  * A_norm factorizes: agg[d] = dinv[d] * sum_{e->d} xd[src_e], xd = dinv*x.
  * pooled needs only z[g] = sum_n C[n,g] * (dinv[n] h[n]) with host-built
    C[n,g] = sum_{e: src=n, dst in g} dinv[dst].  pooled = (z @ W2)/1600 + b2.

HW facts (measured): dma_gather is DESCRIPTOR-rate-limited at ~2.05 ns per
gathered element on 4 SWDGE queues, independent of element size 64B-512B;
indirect-DMA/scatter-add/SBUF-source paths are no faster and don't compose.
So minimize gathered-element count and keep everything else off the
critical path:
  * bf16 quad table (4 nodes x 16 feats = 128B payload in a 256B-stride row,
    quad index src>>2 < 12800 fits int16); elem_size 128B needs a local
    clone of dma_gather without its elem%256 assert (stride is still 256B).
  * (dst, quad) slots are DEDUPED, and nodes are RENUMBERED by a greedy
    co-citation matching (pairs -> quads) so sources cited by the same dst
    share quads: merged slots cut gathered elements ~6% (mask is
    multi-hot, weights accumulate).
  * self-loops never enter the gather: host streams sxd = dinv^2 * x rows
    and the kernel adds them to the aggregation directly.
  * per-slot mask (bf16, dinv[dst] x onehot(src&3)) selects the sub-row
    during a fused DVE multiply + free-axis segment reduce.
  * the whole idx + mask tables fit in SBUF and load once as constants;
    nothing streams against the gathers.  Chunk counts are tuned to a
    multiple of 4 so equal-size gathers round-robin the 4 queues evenly.
Device work per core: ~98.5k-slot gather (~12.6 MB random 128B reads),
mask-mult + segment reduce, transpose, @W1+b1, relu*dinv, z psum-accum,
(z@W2+200*b2)/1600.  Host combines: output = sum of 8 partial p_out.
"""

import numpy as np

N = 51200
E = 819200
F = 16          # input feats
H1 = 64         # hidden
H2 = 128        # output feats
G = 32          # pool groups
GS = N // G     # 1600 nodes per group
NCORES = 8
NPC = N // NCORES       # nodes per core: 6400
NBLK = NPC // 128       # 50 blocks of 128 nodes
PAD_IDX = 10_000_000    # host-side empty-slot marker
GCOLS = 64              # max slot columns per dma_gather (8192 idxs)
CHUNK_G = 2             # gathers per chunk
CHUNK_COLS = 128        # max slot columns per chunk


def _quad_match(src, dst):
    """Renumber nodes so sources co-cited by the same dst share quads:
    co-cited quad members merge into ONE gather slot (the 4-wide mask is
    multi-hot), cutting gathered-element count ~6%.  Greedy maximal
    matching on the co-citation graph (nodes->pairs), then again
    (pairs->quads).  Returns newpos[old_node] = new position."""
    rng = np.random.default_rng(0)
    order = np.argsort(dst, kind="stable")
    ds, ss = dst[order], src[order]

    def pair_events(ids):
        out, k = [], 1
        while True:
            same = ds[:-k] == ds[k:]
            if not same.any():
                break
            a, b = ids[:-k][same], ids[k:][same]
            lo, hi = np.minimum(a, b), np.maximum(a, b)
            ok = lo != hi
            out.append(lo[ok].astype(np.int64) * (1 << 20) + hi[ok])
            k += 1
        return np.concatenate(out)

    def match(u, v, w, nn, used, maxpass=15):
        key = w.astype(np.float64) + rng.random(len(w))
        o = np.argsort(-key, kind="stable")
        u, v = u[o], v[o]
        su_all, sv_all = [], []
        for _ in range(maxpass):
            ok = ~used[u] & ~used[v]
            u, v = u[ok], v[ok]
            if len(u) == 0:
                break
            pos = np.arange(len(u))
            first = np.full(nn, 1 << 60, np.int64)
            np.minimum.at(first, u, pos)
            np.minimum.at(first, v, pos)
            take = (first[u] == pos) & (first[v] == pos)
            used[u[take]] = True
            used[v[take]] = True
            su_all.append(u[take])
            sv_all.append(v[take])
            u, v = u[~take], v[~take]
        return np.concatenate(su_all), np.concatenate(sv_all)

    uq, w = np.unique(pair_events(ss), return_counts=True)
    used = np.zeros(N, bool)
    pu, pv = match(uq >> 20, uq & ((1 << 20) - 1), w, N, used)
    P = len(pu)
    snode = np.full(N, -1, np.int64)
    snode[pu] = np.arange(P)
    snode[pv] = np.arange(P)
    singles = np.where(snode < 0)[0]
    snode[singles] = P + np.arange(len(singles))
    NS = P + len(singles)
    uq2, w2 = np.unique(pair_events(snode[ss]), return_counts=True)
    u2, v2 = uq2 >> 20, uq2 & ((1 << 20) - 1)
    bp = (u2 < P) & (v2 < P)
    used2 = np.zeros(NS, bool)
    qu, qv = match(u2[bp], v2[bp], w2[bp], NS, used2)

    quads = [[pu[a], pv[a], pu[b], pv[b]] for a, b in zip(qu, qv)]
    left = np.where(~used2[:P])[0]
    for i in range(0, len(left) - 1, 2):
        a, b = left[i], left[i + 1]
        quads.append([pu[a], pv[a], pu[b], pv[b]])
    tail = ([pu[left[-1]], pv[left[-1]]] if len(left) % 2 else []) \
        + list(singles)
    for i in range(0, len(tail), 4):
        quads.append(tail[i:i + 4])
    newpos = np.full(N, -1, np.int64)
    p = 0
    for q in quads:
        for n in q:
            newpos[n] = p
            p += 1
    assert p == N and (newpos >= 0).all()
    return newpos


def _prep(x, edge_index, W1, b1, W2, b2):
    """Host-side graph preprocessing: degrees, norms, C matrix, per-core
    deduped quad-slot + mask tables.  Returns (static_cfg, per_core_inmaps)."""
    import ml_dtypes
    bf16 = ml_dtypes.bfloat16

    src = edge_index[0].astype(np.int64)
    dst = edge_index[1].astype(np.int64)

    deg_e = np.bincount(dst, minlength=N)           # edge in-degree
    deg = deg_e + 1                                 # + self loop
    dinv = (1.0 / np.sqrt(deg.astype(np.float64))).astype(np.float32)

    xd = (x.astype(np.float32) * dinv[:, None]).astype(np.float32)
    newpos = _quad_match(src, dst)          # co-citation-aware quad layout
    inv_perm = np.argsort(newpos)           # new position -> old node
    xdq = np.zeros((N // 4 + 1, 8 * F), bf16)       # 256B rows, 128B payload
    xdq[:N // 4, :4 * F] = xd[inv_perm].reshape(N // 4, 4 * F).astype(bf16)

    # C[n, g] = sum_{e: src=n, dst//GS=g} dinv[dst]  (+ self loop term)
    g_e = dst // GS
    C = np.bincount(src * G + g_e, weights=dinv[dst].astype(np.float64),
                    minlength=N * G).astype(np.float32).reshape(N, G)
    C[np.arange(N), np.arange(N) // GS] += dinv

    # deduped (dst, quad) slots with per-subrow accumulated dinv weights
    ps = newpos[src]
    keys = (dst << 14) | (ps >> 2)
    uk, inv = np.unique(keys, return_inverse=True)
    nslot = uk.shape[0]
    mvals = np.zeros((nslot, 4), np.float32)
    np.add.at(mvals, (inv, ps & 3), dinv[dst])
    u_dst = (uk >> 14).astype(np.int64)
    u_quad = (uk & 16383).astype(np.int32)
    nsl = np.bincount(u_dst, minlength=N)           # slots per node
    row_start = np.zeros(N + 1, np.int64)
    np.cumsum(nsl, out=row_start[1:])

    # slot table per node, padded per 128-node block
    maxsl = int(nsl.max())
    Tq = np.full((N, maxsl), N // 4, np.int32)      # pad -> zeros row
    Tm = np.zeros((N, maxsl, 4), np.float32)
    cols = (np.arange(nslot) - row_start[u_dst])
    Tq[u_dst, cols] = u_quad
    Tm[u_dst, cols] = mvals

    # degree-sorted, strided node->core assignment (same cap profile on
    # every core -> one SPMD program)
    order_n = np.argsort(nsl, kind="stable")
    cores_nodes = [order_n[c::NCORES] for c in range(NCORES)]

    caps = []
    for B in range(NBLK):
        m = 1
        for c in range(NCORES):
            nodes = cores_nodes[c][B * 128:(B + 1) * 128]
            m = max(m, int(nsl[nodes].max()))
        caps.append(m)

    # chunk assignment: whole blocks per chunk, <= `bound` slot columns.
    # Pick the largest bound <= CHUNK_COLS whose chunk count is a multiple
    # of 4, so the CHUNK_G equal gathers per chunk round-robin onto the 4
    # SWDGE queues with identical per-queue element totals.
    def mk_chunks(bound):
        ch = []
        col0, b0, acc = 0, 0, 0
        for B in range(NBLK):
            if acc + caps[B] > bound:
                ch.append((b0, B, col0))
                col0 += bound
                b0, acc = B, 0
            acc += caps[B]
        ch.append((b0, NBLK, col0))
        return ch

    cbound = CHUNK_COLS
    for bound in range(CHUNK_COLS, CHUNK_COLS // 2, -1):
        if len(mk_chunks(bound)) % 4 == 0:
            cbound = bound
            break
    chunks = mk_chunks(cbound)
    SP = chunks[-1][2] + cbound     # padded total slot columns
    boff = {}
    for (bb0, bb1, c0) in chunks:
        c = c0
        for B in range(bb0, bb1):
            boff[B] = c
            c += caps[B]

    # block-diagonal W1 (8 copies) so a whole chunk's aggT goes through one
    # matmul; b1 tiled to match
    w1bd = np.zeros((128, 8 * H1), np.float32)
    for b in range(8):
        w1bd[b * F:(b + 1) * F, b * H1:(b + 1) * H1] = W1.astype(np.float32)
    b1rep = np.tile(b1.astype(np.float32), 8).reshape(1, 8 * H1)
    w2 = np.ascontiguousarray(W2.astype(np.float32))
    b2r = np.ascontiguousarray(b2.astype(np.float32).reshape(1, H2))

    W16 = GCOLS * 128 // 16      # 512 idx columns per gather
    per_core = []
    for c in range(NCORES):
        qidx_cols = np.full((128, SP), N // 4, np.int32)
        mask_all = np.zeros((128, SP, 4), np.float32)
        dinv_pos = np.zeros((128, NBLK), np.float32)
        c_all = np.zeros((128, NBLK * G), np.float32)
        sxd = np.zeros((128, NBLK * F), np.float32)
        for B in range(NBLK):
            nodes = cores_nodes[c][B * 128:(B + 1) * 128]
            cap = caps[B]
            o = boff[B]
            qidx_cols[:, o:o + cap] = Tq[nodes, :cap]
            mask_all[:, o:o + cap, :] = Tm[nodes, :cap]
            dinv_pos[:, B] = dinv[nodes]
            # dinv (layer-2 src norm) folded into C: z-matmul needs no
            # per-partition scale, so the relu batches per chunk
            c_all[:, B * G:(B + 1) * G] = C[nodes] * dinv[nodes][:, None]
            sxd[:, B * F:(B + 1) * F] = xd[nodes] * dinv[nodes][:, None]
        # per-gather wrap-16 int16 index streams, replicated to 128 parts.
        # Each chunk's VALID columns (block caps are packed back-to-back
        # from its base) are split into CHUNK_G equal gathers -> all
        # gathers are nearly the same size and round-robin across the 4
        # queues stays balanced at every pipeline instant.
        gathers = []     # (chunk_i, local_colstart, ncols, idx_col_offset)
        parts = []
        icol = 0
        for cidx, (bb0, bb1, cc0) in enumerate(chunks):
            v = boff[bb1 - 1] + caps[bb1 - 1] - cc0   # valid cols in chunk
            s0 = 0
            for gi in range(CHUNK_G):
                ncols = (v + CHUNK_G - 1 - gi) // CHUNK_G
                if ncols == 0:
                    continue
                pos = qidx_cols[:, cc0 + s0:cc0 + s0 + ncols]
                pv = pos.T.ravel()                     # position order
                parts.append(pv.reshape(ncols * 8, 16).T.astype(np.int16))
                gathers.append((cidx, s0, ncols, icol))
                s0 += ncols
                icol += ncols * 8
        gidx = np.tile(np.concatenate(parts, axis=1), (8, 1))
        per_core.append(dict(
            xdq=xdq, gidx=gidx,
            mask_all=mask_all.reshape(128, SP * 4).astype(bf16),
            c_all=c_all, sxd=sxd,
            w1bd=w1bd, b1rep=b1rep, w2=w2, b2r=b2r,
        ))

    cfg = (tuple(caps), tuple(chunks), SP, tuple(gathers), icol, cbound)
    return cfg, per_core


def _small_elem_gather(eng, out_ap, in_ap, idxs_ap, num_idxs, num_idxs_reg,
                       elem_size, elem_step, queue_num):
    """dma_gather clone without the elem_size_bytes%256 assert; the encoded
    stride (elem_step) must still be a 256B multiple."""
    from concourse import ap_utils, mybir
    from concourse.bass import MemorySpace
    eng._assert_queue_num(queue_num)
    assert idxs_ap.dtype == mybir.dt.int16
    assert in_ap.space == MemorySpace.DRAM
    assert in_ap.dtype == out_ap.dtype
    assert ap_utils.ap_is_contiguous(out_ap.ap[1:])
    assert ap_utils.ap_is_contiguous(idxs_ap.ap[1:])
    assert in_ap.ap[-1][1] == out_ap.ap[-1][1] == elem_size
    assert in_ap.ap[0][0] == elem_step
    stride_bytes = elem_step * mybir.dt.size(in_ap.dtype)
    stride_bytes_256 = stride_bytes // 256
    assert stride_bytes % 256 == 0 and stride_bytes_256 < 256
    _in_ap = eng.lower_ap_dma(in_ap, for_custom_bir_dma=True)
    _idxs_ap = eng.lower_ap(idxs_ap)
    _out_ap = eng.lower_ap(out_ap)
    return eng.add_instruction(
        mybir.InstDMAGatherAnt(
            name=eng.bass.get_next_instruction_name(),
            ins=[*_in_ap, _idxs_ap,
                 eng.lower_val_access(eng.to_reg(num_idxs_reg))],
            outs=[_out_ap],
            transpose=False,
            num_idxs=num_idxs,
            elem_size=elem_size,
            stride_bytes_256=stride_bytes_256,
            gen_mode=0,
            single_packet=False,
            queue_num=queue_num,
            sbuf_tokens_per_rank=0,
            sbuf_free_dim_per_rank=0,
            sbuf_free_dim_pad_per_rank=0,
            sbuf_byte_offset=0,
        ))


def _build(cfg, nrep=1, ablate=None):
    # nrep > 1 duplicates the pipeline body (timing only; output invalid).
    # ablate: None | "gather" (skip compute) | "compute" (skip gathers) —
    # timing-only in-situ profiling since NTFF tracing is unavailable.
    import concourse.bass as bass
    import concourse.bacc as bacc
    import concourse.tile as tile
    from concourse import mybir
    from concourse.masks import make_identity

    caps, chunks, SP, gathers, idxcols, cbound = cfg
    boff = {}
    for (bb0, bb1, c0) in chunks:
        c = c0
        for B in range(bb0, bb1):
            boff[B] = c
            c += caps[B]

    f32 = mybir.dt.float32
    bf16 = mybir.dt.bfloat16
    i16 = mybir.dt.int16

    nc = bacc.Bacc("TRN2", target_bir_lowering=False, debug=False,
                   num_devices=NCORES, num_swdge_queues=4)

    xdq_t = nc.dram_tensor("xdq", [N // 4 + 1, 8 * F], bf16,
                           kind="ExternalInput")
    gidx_t = nc.dram_tensor("gidx", [128, idxcols], i16,
                            kind="ExternalInput")
    mask_t = nc.dram_tensor("mask_all", [128, SP * 4], bf16,
                            kind="ExternalInput")
    c_t = nc.dram_tensor("c_all", [128, NBLK * G], f32, kind="ExternalInput")
    sxd_t = nc.dram_tensor("sxd", [128, NBLK * F], f32, kind="ExternalInput")
    w1bd_t = nc.dram_tensor("w1bd", [128, 8 * H1], f32, kind="ExternalInput")
    b1_t = nc.dram_tensor("b1rep", [1, 8 * H1], f32, kind="ExternalInput")
    w2_t = nc.dram_tensor("w2", [H1, H2], f32, kind="ExternalInput")
    b2_t = nc.dram_tensor("b2r", [1, H2], f32, kind="ExternalInput")
    out_t = nc.dram_tensor("p_out", [G, H2], f32, kind="ExternalOutput")

    by_chunk = {}
    for (cidx, s0, ncols, icol) in gathers:
        by_chunk.setdefault(cidx, []).append((s0, ncols, icol))

    AF = mybir.ActivationFunctionType
    AX = mybir.AxisListType
    OP = mybir.AluOpType

    with tile.TileContext(nc) as tc:
        with tc.tile_pool(name="const", bufs=1) as constp, \
             tc.tile_pool(name="stream", bufs=2) as streamp, \
             tc.tile_pool(name="work", bufs=3) as workp, \
             tc.tile_pool(name="psum", bufs=2, space="PSUM") as psump, \
             tc.tile_pool(name="psumacc", bufs=1, space="PSUM") as psumaccp:

            ident = constp.tile([128, 128], f32)
            make_identity(nc, ident[:])
            ones_row = constp.tile([1, 128], f32)
            nc.vector.memset(ones_row[:], 1.0)
            ones_b2 = constp.tile([1, G], f32)
            nc.vector.memset(ones_b2[:], float(GS) / NCORES)   # 200.0

            w1bd = constp.tile([128, 8 * H1], f32)
            nc.sync.dma_start(out=w1bd[:], in_=w1bd_t[:, :])
            b1s = constp.tile([1, 8 * H1], f32)
            nc.sync.dma_start(out=b1s[:], in_=b1_t[:, :])
            w2 = constp.tile([H1, H2], f32)
            nc.sync.dma_start(out=w2[:], in_=w2_t[:, :])
            b2s = constp.tile([1, H2], f32)
            nc.sync.dma_start(out=b2s[:], in_=b2_t[:, :])
            call = constp.tile([128, NBLK * G], f32)
            nc.sync.dma_start(out=call[:], in_=c_t[:, :])
            sxd = constp.tile([128, NBLK * F], f32)
            nc.sync.dma_start(out=sxd[:], in_=sxd_t[:, :])
            # whole idx + mask tables fit in SBUF: load once, no per-rep
            # streaming competes with the gathers
            idxall = constp.tile([128, idxcols], i16)
            nc.sync.dma_start(out=idxall[:], in_=gidx_t[:, :])
            maskall = constp.tile([128, SP * 4], bf16)
            nc.scalar.dma_start(out=maskall[:], in_=mask_t[:, :])

            psum_zT = psumaccp.tile([H1, G], f32)

            gq = 0
            rep_chunks = [c for _ in range(nrep) for c in chunks]
            for ci, (b0, b1_, c0) in enumerate(rep_chunks):
                cidx = ci % len(chunks)
                gchunk = streamp.tile([128, cbound * 4 * F], bf16,
                                      tag="gbuf", bufs=6)
                if ablate == "compute":
                    nc.gpsimd.memset(gchunk[:], 0.0)
                else:
                    for (s0, ncols, icol) in by_chunk[cidx]:
                        _small_elem_gather(
                            nc.gpsimd,
                            out_ap=gchunk[:, s0 * 4 * F:
                                          (s0 + ncols) * 4 * F].rearrange(
                                "p (s f) -> p s f", f=4 * F),
                            in_ap=xdq_t[:, :4 * F],
                            idxs_ap=idxall[:, icol:icol + ncols * 8],
                            num_idxs=ncols * 128,
                            num_idxs_reg=ncols * 128,
                            elem_size=4 * F, elem_step=8 * F,
                            queue_num=gq % 4)
                        gq += 1
                if ablate == "gather":
                    # consume one tiny slice of EACH gather so the final
                    # output depends on every DMA (forces full drain) while
                    # adding negligible DVE/PE work
                    agg = workp.tile([128, F], f32, tag="agg")
                    for (s0, ncols, icol) in by_chunk[cidx]:
                        nc.vector.tensor_reduce(
                            agg[:],
                            gchunk[:, s0 * 4 * F:(s0 + 1) * 4 * F].rearrange(
                                "p (s f) -> p f s", f=F),
                            axis=AX.X, op=OP.add)
                    nc.tensor.matmul(out=psum_zT[:F, :], lhsT=agg[:],
                                     rhs=call[:, :G],
                                     start=(ci == 0),
                                     stop=(ci == len(rep_chunks) - 1),
                                     skip_group_check=True)
                    continue
                for B in range(b0, b1_):
                    o = boff[B] - c0
                    cap = caps[B]
                    gv = gchunk[:, o * 4 * F:(o + cap) * 4 * F].rearrange(
                        "p (c j f) -> p c j f", j=4, f=F)
                    mv = maskall[:, (c0 + o) * 4:(c0 + o + cap) * 4].rearrange(
                        "p (c j) -> p c j", j=4)
                    mvb = bass.AP(mv.tensor, mv.offset,
                                  list(mv.ap) + [[0, F]])
                    # in-place: select the right sub-row and fold dinv[dst]
                    nc.vector.tensor_tensor(out=gv, in0=gv, in1=mvb,
                                            op=OP.mult)
                    agg = workp.tile([128, F], f32, tag="agg")
                    nc.vector.tensor_reduce(
                        agg[:],
                        gchunk[:, o * 4 * F:(o + cap) * 4 * F].rearrange(
                            "p (s f) -> p f s", f=F),
                        axis=AX.X, op=OP.add)
                    # self-loop term, host-prescaled: agg += dinv^2 * x
                    nc.vector.tensor_tensor(
                        out=agg[:], in0=agg[:],
                        in1=sxd[:, B * F:(B + 1) * F], op=OP.add)
                    pt = psump.tile([F, 128], f32, tag="pt")
                    nc.tensor.transpose(out=pt[:], in_=agg[:],
                                        identity=ident[:])
                    aggT = workp.tile([F, 128], f32, tag="aggT")
                    nc.scalar.copy(aggT[:], pt[:])
                    ph = psump.tile([128, H1], f32, tag="ph")
                    nc.tensor.matmul(out=ph[:], lhsT=aggT[:],
                                     rhs=w1bd[:F, :H1],
                                     start=True, stop=False)
                    nc.tensor.matmul(out=ph[:], lhsT=ones_row[:],
                                     rhs=b1s[:, :H1],
                                     start=False, stop=True)
                    hd = workp.tile([128, H1], f32, tag="hd")
                    nc.scalar.activation(hd[:], ph[:], AF.Relu)
                    nc.tensor.matmul(out=psum_zT[:], lhsT=hd[:],
                                     rhs=call[:, B * G:(B + 1) * G],
                                     start=(ci == 0 and B == b0),
                                     stop=(ci == len(rep_chunks) - 1
                                           and B == b1_ - 1),
                                     skip_group_check=True)

            zT = constp.tile([H1, G], f32)
            nc.scalar.copy(zT[:], psum_zT[:])
            pP = psump.tile([G, H2], f32, tag="pP")
            nc.tensor.matmul(out=pP[:], lhsT=zT[:], rhs=w2[:],
                             start=True, stop=False)
            nc.tensor.matmul(out=pP[:], lhsT=ones_b2[:], rhs=b2s[:],
                             start=False, stop=True)
            pout = constp.tile([G, H2], f32)
            nc.scalar.activation(pout[:], pP[:], AF.Copy, scale=1.0 / GS)
            nc.sync.dma_start(out=out_t[:, :], in_=pout[:])

    nc.compile()
    return nc


_CACHE = {}


def kernel(**inputs):
    x = np.asarray(inputs["x"], dtype=np.float32)
    edge_index = np.asarray(inputs["edge_index"])
    W1 = np.asarray(inputs["W1"], dtype=np.float32)
    b1 = np.asarray(inputs["b1"], dtype=np.float32)
    W2 = np.asarray(inputs["W2"], dtype=np.float32)
    b2 = np.asarray(inputs["b2"], dtype=np.float32)
    assert x.shape == (N, F) and edge_index.shape == (2, E)

    cfg, per_core = _prep(x, edge_index, W1, b1, W2, b2)

    from concourse.bass_utils import run_bass_kernel_spmd

    if cfg not in _CACHE:
        _CACHE[cfg] = _build(cfg)
    nc = _CACHE[cfg]

    res = run_bass_kernel_spmd(nc, per_core, list(range(NCORES)))
    out = np.zeros((G, H2), np.float64)
    for r in res.results:
        out += r["p_out"].astype(np.float64)
    return out.astype(np.float32).reshape(1, G, H2)


if __name__ == "__main__":
    rng = np.random.default_rng(0)
    ins = dict(
        x=rng.standard_normal((N, F), dtype=np.float32),
        edge_index=rng.integers(0, N, (2, E)).astype(np.int32),
        W1=rng.standard_normal((F, H1), dtype=np.float32) * 0.25,
        b1=np.zeros(H1, np.float32),
        W2=rng.standard_normal((H1, H2), dtype=np.float32) * 0.125,
        b2=np.zeros(H2, np.float32),
    )
    out = kernel(**ins)
    print(out.shape, out.dtype, float(np.abs(out).mean()))
